# revision 6
# baseline (speedup 1.0000x reference)
"""Causal self-attention (B=4, S=2048, D=1024, H=16) on 8 Trainium2 NeuronCores.

Sharding: core c handles batch b = c // 2 and head-group g = c % 2
(8 heads, 512 of the 1024 output dims).  Data parallel over B, tensor
parallel over heads — attention is embarrassingly parallel over (b, h).

Per-core device program (identical on all cores, SPMD with different data):
  1. Projections: QT/KT in [d, q] layout (d on partitions), V in natural
     [k, d] layout with a ones-column appended (so the P@V matmul also
     produces the softmax denominator as an extra output row).
     All matmul operands fp16 (host-cast), accumulation in fp32 PSUM.
  2. Attention per head-pair: scoresT[k, q] tiles via row-packed (d=64)
     matmuls for two heads concurrently; exp on ScalarE with per-partition
     bias = -SHIFT + attention-mask bias (scale 1/sqrt(64) folded into Wq
     host-side); causal mask via tile skipping + one triangular 128x128
     multiply on diagonal tiles; PV accumulates ctxT[d(+1), q] over k-tiles.
  3. Unnormalized ctxT and the denominator row are DMA'd out (fp16); the
     host divides in fp32 and re-assembles the [B, S, D] output.

Scheduling (the key to overlap): projection work is broken into "steps" of
2 matmuls and streamed into the attention iteration loop, paced per chunk
and deadline-forced, so the PE fills the slack of ACT(exp)-bound stretches.
QT/KT/V are double-buffered by body parity so each repeat body's chunk-0
projections run inside the previous body's ACT-bound chunk-3 tail
(software pipelining across bodies; this is what the repeat-slope measures).
"""

import os

import numpy as np

B, S, D, H, HD = 4, 2048, 1024, 16, 64
DC = 512          # output dims per core (8 heads)
P = 128
NQC = S // 512    # q-chunks of 512
NKT = S // P      # k-tiles of 128
SHIFT = 8.0       # exp(score - SHIFT); cancels in the normalization
NEG = -30000.0    # attention-mask "minus infinity"

_PROG = None


def _chunk_units(nc, t, tp, pools, c):
    """Projection work for q-chunk c (parity tensors tp) as named units.

    Returns dict: ('Q', dt) / ('K', dt) -> 4 steps; ('V', kt_abs) -> 4 steps.
    Each step is a callable emitting 2 matmuls (plus the PSUM-group finish).
    """
    from concourse import mybir
    from concourse.bass import ds, ts

    f32 = mybir.dt.float32
    ADD = mybir.AluOpType.add
    epool, opool, psp, pss, psc = pools
    qsl = ds(c * 512, 512)
    units = {}

    def group(mm_args, fin):
        cell = {}

        def step(i):
            def run():
                if i == 0:
                    cell["pp"] = psp.tile([P, 512], f32, tag="proj", name="pp")
                pp = cell["pp"]
                for s in (2 * i, 2 * i + 1):
                    lhsT, rhs = mm_args(s)
                    nc.tensor.matmul(
                        pp[:], lhsT, rhs, start=(s == 0), stop=(s == 7)
                    )
                if i == 3:
                    fin(pp)
            return run

        return [step(i) for i in range(4)]

    def qk_fin(dst, bt, dt):
        def fin(pp):
            nc.vector.tensor_scalar_add(
                dst[:, dt, qsl], pp[:], bt[:, dt : dt + 1]
            )
        return fin

    def v_fin(kt_i):
        def fin(pp):
            nc.vector.tensor_tensor(
                tp["v65"][:, kt_i, :, 0:64],
                pp[:].rearrange("p (h d) -> p h d", h=8),
                t["bvr_t"][:].rearrange("p (h d) -> p h d", h=8),
                ADD,
            )
        return fin

    for dt in range(4):
        units[("K", dt)] = group(
            lambda s, dt=dt: (t["wkt"][:, s, ts(dt, P)], t["ht"][:, s, qsl]),
            qk_fin(tp["ktt"], t["bk_t"], dt),
        )
        kt_i = 4 * c + dt
        units[("V", kt_i)] = group(
            lambda s, kt_i=kt_i: (
                t["ht"][:, s, ds(kt_i * P, P)], t["wvt"][:, s, :],
            ),
            v_fin(kt_i),
        )
        units[("Q", dt)] = group(
            lambda s, dt=dt: (t["wqt"][:, s, ts(dt, P)], t["ht"][:, s, qsl]),
            qk_fin(tp["qt"], t["bq_t"], dt),
        )
    return units


def _emit_chunk_attention(nc, t, tp, pools, c, queue, rate):
    """Attention for q-chunk c reading parity tensors tp.

    queue: list of (deadline, step_fn); deadline is (pr, kt_i) within this
    chunk ((4, 0) = end of chunk).  Paced at `rate` steps per kt-iteration
    with deadline forcing as the correctness backstop; leftovers drain at
    the chunk end.
    """
    from concourse import mybir
    from concourse.bass import ds

    f32 = mybir.dt.float32
    f16 = mybir.dt.float16
    EXP = mybir.ActivationFunctionType.Exp
    MULT = mybir.AluOpType.mult
    epool, opool, psp, pss, psc = pools

    budget = 0.0
    nkt = 4 * c + 4
    for pr in range(4):
        prate = rate[pr] if isinstance(rate, (list, tuple)) else rate
        cA = psc.tile([P, 512], f32, tag="ctx", name="cA")
        cB = psc.tile([P, 512], f32, tag="ctx", name="cB")
        for kt_i in range(nkt):
            while queue and queue[0][0] <= (pr, kt_i):
                queue.pop(0)[1]()
                budget -= 1.0
            j = kt_i - 4 * c
            off = 128 * j if j > 0 else 0
            qso = ds(c * 512 + off, 512 - off)
            pt = pss.tile([P, 2, 512], f32, tag="sc")
            nc.tensor.matmul(
                pt[:, 0, off:512],
                tp["ktt"][0:64, pr, ds(kt_i * P, P)],
                tp["qt"][0:64, pr, qso],
                start=True, stop=True, tile_position=(0, 0),
            )
            nc.tensor.matmul(
                pt[:, 1, off:512],
                tp["ktt"][64:128, pr, ds(kt_i * P, P)],
                tp["qt"][64:128, pr, qso],
                start=True, stop=True, tile_position=(64, 0),
            )
            e = epool.tile([P, 2, 512], f16, tag="e")
            kbias = t["kb_t"][:, kt_i : kt_i + 1]
            nc.scalar.activation(
                e[:, :, off:512], pt[:, :, off:512], EXP, bias=kbias
            )
            if j >= 0:
                nc.vector.tensor_tensor(
                    e[:, :, off : off + P],
                    e[:, :, off : off + P],
                    t["tri_t"][:][:, None, :].to_broadcast((P, 2, P)),
                    MULT,
                )
            budget += prate
            while budget >= 1.0 and queue:
                queue.pop(0)[1]()
                budget -= 1.0
            nc.tensor.matmul(
                cA[0:65, off:512],
                tp["v65"][:, kt_i, 2 * pr, :], e[:, 0, off:512],
                start=(kt_i == 0), stop=(kt_i == nkt - 1),
            )
            nc.tensor.matmul(
                cB[0:65, off:512],
                tp["v65"][:, kt_i, 2 * pr + 1, :], e[:, 1, off:512],
                start=(kt_i == 0), stop=(kt_i == nkt - 1),
            )
        oA = opool.tile([P, 512], f16, tag="o", name="oA")
        oB = opool.tile([P, 512], f16, tag="o", name="oB")
        nc.vector.tensor_copy(oA[0:65, :], cA[0:65, :])
        nc.vector.tensor_copy(oB[0:65, :], cB[0:65, :])
        qsl = ds(c * 512, 512)
        nc.sync.dma_start(t["out_d"][2 * pr, :, qsl], oA[0:65, :])
        nc.sync.dma_start(t["out_d"][2 * pr + 1, :, qsl], oB[0:65, :])
    while queue:
        queue.pop(0)[1]()


# consumption order of a body's chunk-0 units inside the previous body's
# tail (K/Q of pair 0 and all four V tiles first, then the later pairs)
def _c0_unit_order(base_kt):
    return [
        ("K", 0), ("Q", 0), ("V", base_kt + 0), ("V", base_kt + 1),
        ("V", base_kt + 2), ("V", base_kt + 3),
        ("K", 1), ("Q", 1), ("K", 2), ("Q", 2), ("K", 3), ("Q", 3),
    ]


def _steps(units, order, deadline=(4, 0)):
    """Flatten named units into (deadline, step) entries."""
    out = []
    for key in order:
        for st in units[key]:
            out.append((deadline, st))
    return out


def _emit_bodies(nc, t, tpars, pools, repeat):
    """Emit `repeat` compute bodies with cross-body projection pipelining."""
    n2 = int(os.environ.get("K_N2", "12"))      # next-c0 units placed in c2
    rates = [float(x) for x in os.environ.get(
        "K_RATES", "3.0,1.5,1.1").split(",")]
    # chunk 3 is ACT(exp)-bound: pops must stay under the per-iteration PE
    # slack or they delay the scores->exp spine; front-weighted so the
    # deadline-deferred K/V/Q of chunk 3 land just in time.
    rates3 = [float(x) for x in os.environ.get(
        "K_RATES3", "1.5,0.8,0.5,0.0").split(",")]

    # per-body unit dicts (parity-bound closures)
    units = [
        [_chunk_units(nc, t, tpars[rep % 2], pools, c) for c in range(NQC)]
        for rep in range(repeat)
    ]

    for rep in range(repeat):
        u = units[rep]
        if rep == 0:
            # prologue: body-0 chunk-0 projections in consumption order
            for key in _c0_unit_order(0):
                for st in u[0][key]:
                    st()

        # next body's chunk-0 units (write the other parity: no WAR hazard)
        nxt = units[rep + 1][0] if rep + 1 < repeat else None
        nxt_steps = (
            [st for key in _c0_unit_order(0) for st in nxt[key]]
            if nxt else []
        )
        n2s = min(n2 * 4, len(nxt_steps))

        q0 = _steps(u[1], [k for k in (
            ("K", 0), ("Q", 0), ("V", 4), ("V", 5), ("V", 6), ("V", 7),
            ("K", 1), ("Q", 1), ("K", 2), ("Q", 2), ("K", 3), ("Q", 3))])
        q1 = _steps(u[2], [k for k in (
            ("K", 0), ("Q", 0), ("V", 8), ("V", 9), ("V", 10), ("V", 11),
            ("K", 1), ("Q", 1), ("K", 2), ("Q", 2), ("K", 3), ("Q", 3))])
        # chunk 3's own units: Q d0 goes to the c2 queue (needed at c3
        # start); the rest are deadline-deferred into c3 itself.
        q2 = _steps(u[3], [("Q", 0)]) + [((4, 0), st) for st in
                                         nxt_steps[:n2s]]
        q3 = []
        for key, dl in (
            (("K", 0), (0, 12)), (("V", 12), (0, 12)), (("V", 13), (0, 13)),
            (("V", 14), (0, 14)), (("V", 15), (0, 15)),
            (("Q", 1), (1, 0)), (("K", 1), (1, 12)),
            (("Q", 2), (2, 0)), (("K", 2), (2, 12)),
            (("Q", 3), (3, 0)), (("K", 3), (3, 12)),
        ):
            q3.extend((dl, st) for st in u[3][key])
        q3 += [((4, 0), st) for st in nxt_steps[n2s:]]

        tp = tpars[rep % 2]
        _emit_chunk_attention(nc, t, tp, pools, 0, q0, rates[0])
        _emit_chunk_attention(nc, t, tp, pools, 1, q1, rates[1])
        _emit_chunk_attention(nc, t, tp, pools, 2, q2, rates[2])
        _emit_chunk_attention(nc, t, tp, pools, 3, q3, rates3)


def _build(repeat=1):
    from contextlib import ExitStack

    import concourse.tile as tile
    from concourse import bacc, mybir

    f16, f32 = mybir.dt.float16, mybir.dt.float32

    nc = bacc.Bacc(
        "TRN2",
        target_bir_lowering=False,
        debug=False,
        enable_asserts=False,
        num_devices=8,
    )
    ht_d = nc.dram_tensor("ht", [D, S], f16, kind="ExternalInput").ap()
    wqt_d = nc.dram_tensor("wqt", [D, DC], f16, kind="ExternalInput").ap()
    wkt_d = nc.dram_tensor("wkt", [D, DC], f16, kind="ExternalInput").ap()
    wvt_d = nc.dram_tensor("wvt", [D, DC], f16, kind="ExternalInput").ap()
    bq_d = nc.dram_tensor("bq", [P, 4], f32, kind="ExternalInput").ap()
    bk_d = nc.dram_tensor("bk", [P, 4], f32, kind="ExternalInput").ap()
    bvr_d = nc.dram_tensor("bvr", [P, DC], f16, kind="ExternalInput").ap()
    kb_d = nc.dram_tensor("kbias", [P, NKT], f32, kind="ExternalInput").ap()
    tri_d = nc.dram_tensor("tri", [P, P], f16, kind="ExternalInput").ap()
    out_d = nc.dram_tensor("out", [8, 65, S], f16, kind="ExternalOutput").ap()

    with ExitStack() as ctx:
        tc = ctx.enter_context(tile.TileContext(nc))
        const = ctx.enter_context(tc.tile_pool(name="const", bufs=1))
        epool = ctx.enter_context(
            tc.tile_pool(name="epool", bufs=int(os.environ.get("K_EB", "8")))
        )
        opool = ctx.enter_context(tc.tile_pool(name="opool", bufs=4))
        pss = ctx.enter_context(tc.tile_pool(name="pss", bufs=2, space="PSUM"))
        psp = ctx.enter_context(tc.tile_pool(name="psp", bufs=2, space="PSUM"))
        psc = ctx.enter_context(tc.tile_pool(name="psc", bufs=2, space="PSUM"))

        t = dict(
            ht=const.tile([P, 8, S], f16, name="ht"),
            wqt=const.tile([P, 8, DC], f16, name="wqt"),
            wkt=const.tile([P, 8, DC], f16, name="wkt"),
            wvt=const.tile([P, 8, DC], f16, name="wvt"),
            bq_t=const.tile([P, 4], f32, name="bq_t"),
            bk_t=const.tile([P, 4], f32, name="bk_t"),
            bvr_t=const.tile([P, DC], f16, name="bvr_t"),
            kb_t=const.tile([P, NKT], f32, name="kb_t"),
            tri_t=const.tile([P, P], f16, name="tri_t"),
            out_d=out_d,
        )
        tpars = [
            dict(
                qt=const.tile([P, 4, S], f16, name=f"qt{p}"),
                ktt=const.tile([P, 4, S], f16, name=f"ktt{p}"),
                v65=const.tile([P, NKT, 8, 65], f16, name=f"v65_{p}"),
            )
            for p in range(2)
        ]

        nc.sync.dma_start(t["bq_t"][:], bq_d)
        nc.sync.dma_start(t["bk_t"][:], bk_d)
        nc.sync.dma_start(t["bvr_t"][:], bvr_d)
        nc.sync.dma_start(t["kb_t"][:], kb_d)
        nc.sync.dma_start(t["tri_t"][:], tri_d)
        # warmup exp so the ACT table load (~2.7us) hides behind startup DMAs
        warm = const.tile([P, 1], mybir.dt.float16, name="warm")
        nc.scalar.activation(
            warm[:], t["bq_t"][:, 0:1], mybir.ActivationFunctionType.Exp
        )
        # warm the PE (HAM clock gate) with dummy matmuls while the first
        # input DMAs are still in flight, so real work starts at 2.4 GHz
        scr = const.tile([P, 512], f16, name="scr")
        nc.vector.memset(scr[:], 0.0)
        wpt = psp.tile([P, 512], f32, tag="proj", name="wpt")
        for i in range(14):
            nc.tensor.matmul(
                wpt[:, 0:512], scr[:, 0:P], scr[:],
                start=(i == 0), stop=(i == 13),
            )
        ht_r = ht_d.rearrange("(o p) m -> p o m", p=P)
        wq_r = wqt_d.rearrange("(o p) m -> p o m", p=P)
        wk_r = wkt_d.rearrange("(o p) m -> p o m", p=P)
        wv_r = wvt_d.rearrange("(o p) m -> p o m", p=P)
        # Load order: what q-chunk 0 needs first, so compute starts ASAP.
        for s in range(8):
            nc.sync.dma_start(t["wqt"][:, s, :], wq_r[:, s, :])
            nc.sync.dma_start(t["ht"][:, s, 0:512], ht_r[:, s, 0:512])
        for s in range(8):
            nc.sync.dma_start(t["wkt"][:, s, :], wk_r[:, s, :])
            nc.sync.dma_start(t["wvt"][:, s, :], wv_r[:, s, :])
        for s in range(8):
            nc.sync.dma_start(t["ht"][:, s, 512:2048], ht_r[:, s, 512:2048])
        for p in range(2):
            nc.vector.memset(tpars[p]["v65"][:, :, :, 64:65], 1.0)

        pools = (epool, opool, psp, pss, psc)
        _emit_bodies(nc, t, tpars, pools, repeat)

    nc.compile()
    return nc


def _get_program():
    global _PROG
    if _PROG is None:
        _PROG = _build()
    return _PROG


def prepare_in_maps(hidden_states, attention_mask, Wq, bq, Wk, bk, Wv, bv):
    hidden_states = np.asarray(hidden_states, dtype=np.float32)
    attention_mask = np.asarray(attention_mask)
    Wq, bq = np.asarray(Wq, np.float32), np.asarray(bq, np.float32)
    Wk, bk = np.asarray(Wk, np.float32), np.asarray(bk, np.float32)
    Wv, bv = np.asarray(Wv, np.float32), np.asarray(bv, np.float32)
    tri = np.triu(np.ones((P, P), np.float16))  # tri[k, q] = 1 iff q >= k
    in_maps = []
    hts = [np.ascontiguousarray(hidden_states[b].T, dtype=np.float16)
           for b in range(B)]
    for c in range(8):
        b, g = divmod(c, 2)
        rows = slice(g * DC, (g + 1) * DC)
        am = np.asarray(attention_mask[b, 0, 0], np.float32)
        kbias = (np.where(am > 0, 0.0, NEG) - SHIFT).astype(np.float32)
        in_maps.append(
            dict(
                ht=hts[b],
                wqt=np.ascontiguousarray((Wq[rows] * 0.125).T, np.float16),
                wkt=np.ascontiguousarray(Wk[rows].T, np.float16),
                wvt=np.ascontiguousarray(Wv[rows].T, np.float16),
                bq=np.ascontiguousarray((bq[rows] * 0.125).reshape(4, P).T),
                bk=np.ascontiguousarray(bk[rows].reshape(4, P).T),
                bvr=np.broadcast_to(
                    bv[rows].astype(np.float16), (P, DC)
                ).copy(),
                kbias=np.ascontiguousarray(kbias.reshape(NKT, P).T),
                tri=tri,
            )
        )
    return in_maps


def _assemble(results):
    out = np.empty((B, S, D), np.float32)
    for c in range(8):
        b, g = divmod(c, 2)
        o = results[c]["out"].astype(np.float32)  # [8, 65, S]: ctxT + denom
        ctx = o[:, :64, :] / o[:, 64:65, :]
        out[b, :, g * DC : (g + 1) * DC] = ctx.transpose(2, 0, 1).reshape(S, DC)
    return out


def _run(in_maps, trace=False):
    from concourse.bass_utils import run_bass_kernel_spmd

    nc = _get_program()
    return run_bass_kernel_spmd(nc, in_maps, core_ids=list(range(8)), trace=trace)


def kernel(hidden_states, attention_mask, Wq, bq, Wk, bk, Wv, bv):
    in_maps = prepare_in_maps(
        hidden_states, attention_mask, Wq, bq, Wk, bk, Wv, bv
    )
    res = _run(in_maps, trace=False)
    return _assemble(res.results)


# revision 8
# speedup vs baseline: 1.0277x; 1.0277x over previous
"""Causal self-attention (B=4, S=2048, D=1024, H=16) on 8 Trainium2 NeuronCores.

Sharding: core c handles batch b = c // 2 and head-group g = c % 2
(8 heads, 512 of the 1024 output dims).  Data parallel over B, tensor
parallel over heads — attention is embarrassingly parallel over (b, h).

Per-core device program (identical on all cores, SPMD with different data):
  1. Projections: QT/KT in [d, q] layout (d on partitions), V in natural
     [k, d] layout with a ones-column appended (so the P@V matmul also
     produces the softmax denominator as an extra output row).
     All matmul operands fp16 (host-cast), accumulation in fp32 PSUM.
  2. Attention per head-pair: scoresT[k, q] tiles via row-packed (d=64)
     matmuls for two heads concurrently; exp on ScalarE with per-partition
     bias = -SHIFT + attention-mask bias (scale 1/sqrt(64) folded into Wq
     host-side); causal mask via tile skipping + one triangular 128x128
     multiply on diagonal tiles; PV accumulates ctxT[d(+1), q] over k-tiles.
  3. Unnormalized ctxT and the denominator row are DMA'd out (fp16); the
     host divides in fp32 and re-assembles the [B, S, D] output.

Scheduling (the key to overlap): projection work is broken into "steps" of
2 matmuls and streamed into the attention iteration loop, paced per chunk
and deadline-forced, so the PE fills the slack of ACT(exp)-bound stretches.
QT/KT/V are double-buffered by body parity so each repeat body's chunk-0
projections run inside the previous body's ACT-bound chunk-3 tail
(software pipelining across bodies; this is what the repeat-slope measures).
"""

import os

import numpy as np

B, S, D, H, HD = 4, 2048, 1024, 16, 64
DC = 512          # output dims per core (8 heads)
P = 128
NQC = S // 512    # q-chunks of 512
NKT = S // P      # k-tiles of 128
SHIFT = 8.0       # exp(score - SHIFT); cancels in the normalization
NEG = -30000.0    # attention-mask "minus infinity"

_PROG = None


def _chunk_units(nc, t, tp, pools, c):
    """Projection work for q-chunk c (parity tensors tp) as named units.

    Returns dict: ('Q', dt) / ('K', dt) -> 4 steps; ('V', kt_abs) -> 4 steps.
    Each step is a callable emitting 2 matmuls (plus the PSUM-group finish).
    """
    from concourse import mybir
    from concourse.bass import ds, ts

    f32 = mybir.dt.float32
    ADD = mybir.AluOpType.add
    epool, opool, psp, pss, psc = pools
    qsl = ds(c * 512, 512)
    units = {}

    def group(mm_args, fin):
        cell = {}

        def step(i):
            def run():
                if i == 0:
                    cell["pp"] = psp.tile([P, 512], f32, tag="proj", name="pp")
                pp = cell["pp"]
                for s in (2 * i, 2 * i + 1):
                    lhsT, rhs = mm_args(s)
                    nc.tensor.matmul(
                        pp[:], lhsT, rhs, start=(s == 0), stop=(s == 7)
                    )
                if i == 3:
                    fin(pp)
            return run

        return [step(i) for i in range(4)]

    def qk_fin(dst, bt, dt):
        def fin(pp):
            nc.vector.tensor_scalar_add(
                dst[:, dt, qsl], pp[:], bt[:, dt : dt + 1]
            )
        return fin

    def v_fin(kt_i):
        def fin(pp):
            nc.vector.tensor_tensor(
                tp["v65"][:, kt_i, :, 0:64],
                pp[:].rearrange("p (h d) -> p h d", h=8),
                t["bvr_t"][:].rearrange("p (h d) -> p h d", h=8),
                ADD,
            )
        return fin

    for dt in range(4):
        units[("K", dt)] = group(
            lambda s, dt=dt: (t["wkt"][:, s, ts(dt, P)], t["ht"][:, s, qsl]),
            qk_fin(tp["ktt"], t["bk_t"], dt),
        )
        kt_i = 4 * c + dt
        units[("V", kt_i)] = group(
            lambda s, kt_i=kt_i: (
                t["ht"][:, s, ds(kt_i * P, P)], t["wvt"][:, s, :],
            ),
            v_fin(kt_i),
        )
        units[("Q", dt)] = group(
            lambda s, dt=dt: (t["wqt"][:, s, ts(dt, P)], t["ht"][:, s, qsl]),
            qk_fin(tp["qt"], t["bq_t"], dt),
        )
    return units


def _emit_chunk_attention(nc, t, tp, pools, c, queue, rate):
    """Attention for q-chunk c reading parity tensors tp.

    queue: list of (deadline, step_fn); deadline is (pr, kt_i) within this
    chunk ((4, 0) = end of chunk).  Paced at `rate` steps per kt-iteration
    with deadline forcing as the correctness backstop; leftovers drain at
    the chunk end.
    """
    from concourse import mybir
    from concourse.bass import ds

    f32 = mybir.dt.float32
    f16 = mybir.dt.float16
    EXP = mybir.ActivationFunctionType.Exp
    MULT = mybir.AluOpType.mult
    epool, opool, psp, pss, psc = pools

    budget = 0.0
    nkt = 4 * c + 4
    for pr in range(4):
        prate = rate[pr] if isinstance(rate, (list, tuple)) else rate
        cA = psc.tile([P, 512], f32, tag="ctx", name="cA")
        cB = psc.tile([P, 512], f32, tag="ctx", name="cB")
        for kt_i in range(nkt):
            # all pops sit BEFORE this iteration's scores: the in-order PE
            # stalls at S(kt) on its pss slot (freed by exp(kt-2)) in
            # ACT-bound stretches, and only work emitted ahead of S can
            # fill that gap.
            while queue and queue[0][0] <= (pr, kt_i):
                queue.pop(0)[1]()
                budget -= 1.0
            budget += prate
            while budget >= 1.0 and queue:
                queue.pop(0)[1]()
                budget -= 1.0
            j = kt_i - 4 * c
            off = 128 * j if j > 0 else 0
            qso = ds(c * 512 + off, 512 - off)
            pt = pss.tile([P, 2, 512], f32, tag="sc")
            nc.tensor.matmul(
                pt[:, 0, off:512],
                tp["ktt"][0:64, pr, ds(kt_i * P, P)],
                tp["qt"][0:64, pr, qso],
                start=True, stop=True, tile_position=(0, 0),
            )
            nc.tensor.matmul(
                pt[:, 1, off:512],
                tp["ktt"][64:128, pr, ds(kt_i * P, P)],
                tp["qt"][64:128, pr, qso],
                start=True, stop=True, tile_position=(64, 0),
            )
            e = epool.tile([P, 2, 512], f16, tag="e")
            kbias = t["kb_t"][:, kt_i : kt_i + 1]
            nc.scalar.activation(
                e[:, :, off:512], pt[:, :, off:512], EXP, bias=kbias
            )
            if j >= 0:
                nc.vector.tensor_tensor(
                    e[:, :, off : off + P],
                    e[:, :, off : off + P],
                    t["tri_t"][:][:, None, :].to_broadcast((P, 2, P)),
                    MULT,
                )
            nc.tensor.matmul(
                cA[0:65, off:512],
                tp["v65"][:, kt_i, 2 * pr, :], e[:, 0, off:512],
                start=(kt_i == 0), stop=(kt_i == nkt - 1),
            )
            nc.tensor.matmul(
                cB[0:65, off:512],
                tp["v65"][:, kt_i, 2 * pr + 1, :], e[:, 1, off:512],
                start=(kt_i == 0), stop=(kt_i == nkt - 1),
            )
        oA = opool.tile([P, 512], f16, tag="o", name="oA")
        oB = opool.tile([P, 512], f16, tag="o", name="oB")
        nc.vector.tensor_copy(oA[0:65, :], cA[0:65, :])
        nc.vector.tensor_copy(oB[0:65, :], cB[0:65, :])
        qsl = ds(c * 512, 512)
        nc.sync.dma_start(t["out_d"][2 * pr, :, qsl], oA[0:65, :])
        nc.sync.dma_start(t["out_d"][2 * pr + 1, :, qsl], oB[0:65, :])
    while queue:
        queue.pop(0)[1]()


# consumption order of a body's chunk-0 units inside the previous body's
# tail (K/Q of pair 0 and all four V tiles first, then the later pairs)
def _c0_unit_order(base_kt):
    return [
        ("K", 0), ("Q", 0), ("V", base_kt + 0), ("V", base_kt + 1),
        ("V", base_kt + 2), ("V", base_kt + 3),
        ("K", 1), ("Q", 1), ("K", 2), ("Q", 2), ("K", 3), ("Q", 3),
    ]


def _steps(units, order, deadline=(4, 0)):
    """Flatten named units into (deadline, step) entries."""
    out = []
    for key in order:
        for st in units[key]:
            out.append((deadline, st))
    return out


def _emit_bodies(nc, t, tpars, pools, repeat):
    """Emit `repeat` compute bodies with cross-body projection pipelining."""
    n2 = int(os.environ.get("K_N2", "12"))      # next-c0 units placed in c2
    rates = [float(x) for x in os.environ.get(
        "K_RATES", "3.0,1.5,1.1").split(",")]
    # chunk 3 is ACT(exp)-bound: pops must stay under the per-iteration PE
    # slack or they delay the scores->exp spine; front-weighted so the
    # deadline-deferred K/V/Q of chunk 3 land just in time.
    rates3 = [float(x) for x in os.environ.get(
        "K_RATES3", "1.5,0.8,0.5,0.0").split(",")]

    # per-body unit dicts (parity-bound closures)
    units = [
        [_chunk_units(nc, t, tpars[rep % 2], pools, c) for c in range(NQC)]
        for rep in range(repeat)
    ]

    for rep in range(repeat):
        u = units[rep]
        if rep == 0:
            # prologue: body-0 chunk-0 projections in consumption order
            for key in _c0_unit_order(0):
                for st in u[0][key]:
                    st()

        # next body's chunk-0 units (write the other parity: no WAR hazard)
        nxt = units[rep + 1][0] if rep + 1 < repeat else None
        nxt_steps = (
            [st for key in _c0_unit_order(0) for st in nxt[key]]
            if nxt else []
        )
        n2s = min(n2 * 4, len(nxt_steps))

        q0 = _steps(u[1], [k for k in (
            ("K", 0), ("Q", 0), ("V", 4), ("V", 5), ("V", 6), ("V", 7),
            ("K", 1), ("Q", 1), ("K", 2), ("Q", 2), ("K", 3), ("Q", 3))])
        q1 = _steps(u[2], [k for k in (
            ("K", 0), ("Q", 0), ("V", 8), ("V", 9), ("V", 10), ("V", 11),
            ("K", 1), ("Q", 1), ("K", 2), ("Q", 2), ("K", 3), ("Q", 3))])
        # chunk 3's own units: Q d0 goes to the c2 queue (needed at c3
        # start); the rest are deadline-deferred into c3 itself.
        q2 = _steps(u[3], [("Q", 0)]) + [((4, 0), st) for st in
                                         nxt_steps[:n2s]]
        q3 = []
        for key, dl in (
            (("K", 0), (0, 12)), (("V", 12), (0, 12)), (("V", 13), (0, 13)),
            (("V", 14), (0, 14)), (("V", 15), (0, 15)),
            (("Q", 1), (1, 0)), (("K", 1), (1, 12)),
            (("Q", 2), (2, 0)), (("K", 2), (2, 12)),
            (("Q", 3), (3, 0)), (("K", 3), (3, 12)),
        ):
            q3.extend((dl, st) for st in u[3][key])
        q3 += [((4, 0), st) for st in nxt_steps[n2s:]]

        tp = tpars[rep % 2]
        _emit_chunk_attention(nc, t, tp, pools, 0, q0, rates[0])
        _emit_chunk_attention(nc, t, tp, pools, 1, q1, rates[1])
        _emit_chunk_attention(nc, t, tp, pools, 2, q2, rates[2])
        _emit_chunk_attention(nc, t, tp, pools, 3, q3, rates3)


def _build(repeat=1):
    from contextlib import ExitStack

    import concourse.tile as tile
    from concourse import bacc, mybir

    f16, f32 = mybir.dt.float16, mybir.dt.float32

    nc = bacc.Bacc(
        "TRN2",
        target_bir_lowering=False,
        debug=False,
        enable_asserts=False,
        num_devices=8,
    )
    ht_d = nc.dram_tensor("ht", [D, S], f16, kind="ExternalInput").ap()
    wqt_d = nc.dram_tensor("wqt", [D, DC], f16, kind="ExternalInput").ap()
    wkt_d = nc.dram_tensor("wkt", [D, DC], f16, kind="ExternalInput").ap()
    wvt_d = nc.dram_tensor("wvt", [D, DC], f16, kind="ExternalInput").ap()
    bq_d = nc.dram_tensor("bq", [P, 4], f32, kind="ExternalInput").ap()
    bk_d = nc.dram_tensor("bk", [P, 4], f32, kind="ExternalInput").ap()
    bvr_d = nc.dram_tensor("bvr", [P, DC], f16, kind="ExternalInput").ap()
    kb_d = nc.dram_tensor("kbias", [P, NKT], f32, kind="ExternalInput").ap()
    tri_d = nc.dram_tensor("tri", [P, P], f16, kind="ExternalInput").ap()
    out_d = nc.dram_tensor("out", [8, 65, S], f16, kind="ExternalOutput").ap()

    with ExitStack() as ctx:
        tc = ctx.enter_context(tile.TileContext(nc))
        const = ctx.enter_context(tc.tile_pool(name="const", bufs=1))
        epool = ctx.enter_context(
            tc.tile_pool(name="epool", bufs=int(os.environ.get("K_EB", "8")))
        )
        opool = ctx.enter_context(tc.tile_pool(name="opool", bufs=4))
        pss = ctx.enter_context(tc.tile_pool(name="pss", bufs=2, space="PSUM"))
        psp = ctx.enter_context(tc.tile_pool(name="psp", bufs=2, space="PSUM"))
        psc = ctx.enter_context(tc.tile_pool(name="psc", bufs=2, space="PSUM"))

        t = dict(
            ht=const.tile([P, 8, S], f16, name="ht"),
            wqt=const.tile([P, 8, DC], f16, name="wqt"),
            wkt=const.tile([P, 8, DC], f16, name="wkt"),
            wvt=const.tile([P, 8, DC], f16, name="wvt"),
            bq_t=const.tile([P, 4], f32, name="bq_t"),
            bk_t=const.tile([P, 4], f32, name="bk_t"),
            bvr_t=const.tile([P, DC], f16, name="bvr_t"),
            kb_t=const.tile([P, NKT], f32, name="kb_t"),
            tri_t=const.tile([P, P], f16, name="tri_t"),
            out_d=out_d,
        )
        tpars = [
            dict(
                qt=const.tile([P, 4, S], f16, name=f"qt{p}"),
                ktt=const.tile([P, 4, S], f16, name=f"ktt{p}"),
                v65=const.tile([P, NKT, 8, 65], f16, name=f"v65_{p}"),
            )
            for p in range(2)
        ]

        nc.sync.dma_start(t["bq_t"][:], bq_d)
        nc.sync.dma_start(t["bk_t"][:], bk_d)
        nc.sync.dma_start(t["bvr_t"][:], bvr_d)
        nc.sync.dma_start(t["kb_t"][:], kb_d)
        nc.sync.dma_start(t["tri_t"][:], tri_d)
        # warmup exp so the ACT table load (~2.7us) hides behind startup DMAs
        warm = const.tile([P, 1], mybir.dt.float16, name="warm")
        nc.scalar.activation(
            warm[:], t["bq_t"][:, 0:1], mybir.ActivationFunctionType.Exp
        )
        # warm the PE (HAM clock gate) with dummy matmuls while the first
        # input DMAs are still in flight, so real work starts at 2.4 GHz
        scr = const.tile([P, 512], f16, name="scr")
        nc.vector.memset(scr[:], 0.0)
        wpt = psp.tile([P, 512], f32, tag="proj", name="wpt")
        for i in range(14):
            nc.tensor.matmul(
                wpt[:, 0:512], scr[:, 0:P], scr[:],
                start=(i == 0), stop=(i == 13),
            )
        ht_r = ht_d.rearrange("(o p) m -> p o m", p=P)
        wq_r = wqt_d.rearrange("(o p) m -> p o m", p=P)
        wk_r = wkt_d.rearrange("(o p) m -> p o m", p=P)
        wv_r = wvt_d.rearrange("(o p) m -> p o m", p=P)
        # Load order: what q-chunk 0 needs first, so compute starts ASAP.
        for s in range(8):
            nc.sync.dma_start(t["wqt"][:, s, :], wq_r[:, s, :])
            nc.sync.dma_start(t["ht"][:, s, 0:512], ht_r[:, s, 0:512])
        for s in range(8):
            nc.sync.dma_start(t["wkt"][:, s, :], wk_r[:, s, :])
            nc.sync.dma_start(t["wvt"][:, s, :], wv_r[:, s, :])
        for s in range(8):
            nc.sync.dma_start(t["ht"][:, s, 512:2048], ht_r[:, s, 512:2048])
        for p in range(2):
            nc.vector.memset(tpars[p]["v65"][:, :, :, 64:65], 1.0)

        pools = (epool, opool, psp, pss, psc)
        _emit_bodies(nc, t, tpars, pools, repeat)

    nc.compile()
    return nc


def _get_program():
    global _PROG
    if _PROG is None:
        _PROG = _build()
    return _PROG


def prepare_in_maps(hidden_states, attention_mask, Wq, bq, Wk, bk, Wv, bv):
    hidden_states = np.asarray(hidden_states, dtype=np.float32)
    attention_mask = np.asarray(attention_mask)
    Wq, bq = np.asarray(Wq, np.float32), np.asarray(bq, np.float32)
    Wk, bk = np.asarray(Wk, np.float32), np.asarray(bk, np.float32)
    Wv, bv = np.asarray(Wv, np.float32), np.asarray(bv, np.float32)
    tri = np.triu(np.ones((P, P), np.float16))  # tri[k, q] = 1 iff q >= k
    in_maps = []
    hts = [np.ascontiguousarray(hidden_states[b].T, dtype=np.float16)
           for b in range(B)]
    for c in range(8):
        b, g = divmod(c, 2)
        rows = slice(g * DC, (g + 1) * DC)
        am = np.asarray(attention_mask[b, 0, 0], np.float32)
        kbias = (np.where(am > 0, 0.0, NEG) - SHIFT).astype(np.float32)
        in_maps.append(
            dict(
                ht=hts[b],
                wqt=np.ascontiguousarray((Wq[rows] * 0.125).T, np.float16),
                wkt=np.ascontiguousarray(Wk[rows].T, np.float16),
                wvt=np.ascontiguousarray(Wv[rows].T, np.float16),
                bq=np.ascontiguousarray((bq[rows] * 0.125).reshape(4, P).T),
                bk=np.ascontiguousarray(bk[rows].reshape(4, P).T),
                bvr=np.broadcast_to(
                    bv[rows].astype(np.float16), (P, DC)
                ).copy(),
                kbias=np.ascontiguousarray(kbias.reshape(NKT, P).T),
                tri=tri,
            )
        )
    return in_maps


def _assemble(results):
    out = np.empty((B, S, D), np.float32)
    for c in range(8):
        b, g = divmod(c, 2)
        o = results[c]["out"].astype(np.float32)  # [8, 65, S]: ctxT + denom
        ctx = o[:, :64, :] / o[:, 64:65, :]
        out[b, :, g * DC : (g + 1) * DC] = ctx.transpose(2, 0, 1).reshape(S, DC)
    return out


def _run(in_maps, trace=False):
    from concourse.bass_utils import run_bass_kernel_spmd

    nc = _get_program()
    return run_bass_kernel_spmd(nc, in_maps, core_ids=list(range(8)), trace=trace)


def kernel(hidden_states, attention_mask, Wq, bq, Wk, bk, Wv, bv):
    in_maps = prepare_in_maps(
        hidden_states, attention_mask, Wq, bq, Wk, bk, Wv, bv
    )
    res = _run(in_maps, trace=False)
    return _assemble(res.results)
